# revision 2
# baseline (speedup 1.0000x reference)
"""Trainium2 Bass kernel for nn_CentralizedCritic (pooling critic net).

Data-parallel over 8 NeuronCores: each core handles B_c=2048 batch rows.

Per-core math (matches the jax reference):
  robot_emb = setenc(robot[b], rw*)  -> [B,32]   (mean+max pool over 64)
  track_emb = setenc(track[b], tw*)  -> [B,32]   (mean+max pool over 128)
  c = [tier0, robot_emb, track_emb]  -> [B,108]
  y = mlp(c)                         -> [B]

On-chip mapping:
  - Activations kept transposed [feat, rows]; 2 batch-halves packed on the
    partition dim via block-diag weights (K=2*d_in, M=2*d_hidden=128).
  - x^T pre-packed on host into 4 row-groups at partition offsets {0,32,64,96}
    so L1 matmuls row-tile the PE array.
  - L1/L2/head matmuls in float32r (full-rate fp32), L3 in bf16 so two
    column-tiles can pair into one [128,*] PSUM tile (halves pooling cost).
  - relu+bias fused into the PSUM->SBUF evacuation (ACT or DVE).
  - mean-pool: DVE reduce_sum from PSUM; max-pool: DVE reduce_max from PSUM.
  - e-bias (rb3/tb3) folded into the head-L1 bias on host.
"""

import sys

sys.path.insert(0, "/opt/trn_rl_repo")

import numpy as np
import ml_dtypes

import concourse.bass as bass  # noqa: F401  (bass must import before tile)
import concourse.mybir as mybir
import concourse.tile as tile
from concourse import bacc
from concourse.bass_utils import run_bass_kernel_spmd

F32R = mybir.dt.float32r
F32 = mybir.dt.float32
BF16 = mybir.dt.bfloat16
AF = mybir.ActivationFunctionType
ALU = mybir.AluOpType
AX = mybir.AxisListType

N_CORES = 8
B = 16384
B_C = B // N_CORES          # 2048 batch rows per core
HALF = B_C // 2             # 1024 (2-row packing pairs b and b+HALF)
NR, DR = 64, 6              # robot set size / feature dim
NT, DT = 128, 7             # track set size / feature dim
CT = B_C * NT // 2          # 131072 packed track cols per core
CR = B_C * NR // 2          # 65536 packed robot cols per core
QT = CT // 4                # 32768 cols per track row-group
QR = CR // 4                # 16384 cols per robot row-group
CHUNK = 4096                # dma chunk cols
NTILE = 512                 # matmul free dim

# const-block column layout in "wts" [128, 840] (f32r)
W1T_C, W1R_C, W2T_C, W2R_C = 0, 128, 256, 384
MW1_C, MW2_C, MW3_C, MW4_C = 512, 640, 768, 832
WTS_W = 840
# "bs" [128, 8] (f32) bias columns
BS_TB1, BS_RB1, BS_TB2, BS_RB2, BS_MB1, BS_MB2, BS_MB3, BS_MB4 = range(8)

_CACHE = {}


def _build_bass():
    nc = bacc.Bacc("TRN2", target_bir_lowering=False, debug=False,
                   num_devices=N_CORES)
    xt_d = nc.dram_tensor("xt", [128, QT], F32R, kind="ExternalInput")
    xr_d = nc.dram_tensor("xr", [128, QR], F32R, kind="ExternalInput")
    t0_d = nc.dram_tensor("t0", [44, B_C], F32R, kind="ExternalInput")
    wts_d = nc.dram_tensor("wts", [128, WTS_W], F32R, kind="ExternalInput")
    wbf_d = nc.dram_tensor("wbf", [128, 128], BF16, kind="ExternalInput")
    bs_d = nc.dram_tensor("bs", [128, 8], F32, kind="ExternalInput")
    y_d = nc.dram_tensor("y", [1, B_C], F32, kind="ExternalOutput")

    with tile.TileContext(nc) as tc:
        with (
            tc.tile_pool(name="consts", bufs=1) as consts,
            tc.tile_pool(name="xchunks", bufs=3) as xchunks,
            tc.tile_pool(name="hbuf", bufs=3) as hbuf,
            tc.tile_pool(name="acc", bufs=1) as acc,
            tc.tile_pool(name="head", bufs=2) as head,
            tc.tile_pool(name="ps", bufs=2, space="PSUM") as ps,
        ):
            wts = consts.tile([128, WTS_W], F32R)
            wbf = consts.tile([128, 128], BF16)
            bs = consts.tile([128, 8], F32)
            nc.sync.dma_start(out=wts[:], in_=wts_d[:])
            nc.sync.dma_start(out=wbf[:], in_=wbf_d[:])
            nc.sync.dma_start(out=bs[:], in_=bs_d[:])

            cT = acc.tile([108, B_C], F32R)
            nc.sync.dma_start(out=cT[0:44, :], in_=t0_d[:])

            # pooled accumulators: col = 256*J + 32*ch + 4*t + om (track)
            #                      col = 256*J + 64*ch + 8*t + om (robot)
            esum_t = acc.tile([128, 512], F32, tag="esum_t")
            emax_t = acc.tile([128, 512], F32, tag="emax_t")
            esum_r = acc.tile([128, 512], F32, tag="esum_r")
            emax_r = acc.tile([128, 512], F32, tag="emax_r")

            it_counter = [0]

            def branch(x_d, qcols, k2, w1_c, w2_c, w3_c, bs1, bs2, nseg,
                       esum, emax):
                """One set-encoder branch. k2 = packed K (14 track / 12 robot).
                nseg = set size (128 track / 64 robot)."""
                nchunks = qcols // CHUNK
                tpc = CHUNK // NTILE  # tiles per chunk per group = 8
                for ch in range(nchunks):
                    xc = xchunks.tile([128, CHUNK], F32R, tag="xc")
                    nc.sync.dma_start(
                        out=xc[:], in_=x_d[:, ch * CHUNK:(ch + 1) * CHUNK])
                    for t in range(tpc):
                        it = it_counter[0]
                        it_counter[0] += 1
                        cs = slice(t * NTILE, (t + 1) * NTILE)
                        # ---- L1: 4 row-tiled matmuls (K=k2 each) ----
                        ps1 = ps.tile([128, 4, NTILE], F32, tag="ps")
                        for q in range(4):
                            nc.tensor.matmul(
                                ps1[:, q, :],
                                wts[32 * q:32 * q + k2, w1_c:w1_c + 128],
                                xc[32 * q:32 * q + k2, cs],
                                start=True, stop=True,
                                tile_position=(32 * q, 0),
                            )
                        h1 = hbuf.tile([128, 4, NTILE], F32R, tag="h1")
                        p1v = ps1.rearrange("p a b -> p (a b)")
                        h1v = h1.rearrange("p a b -> p (a b)")
                        nc.scalar.activation(out=h1v[:], in_=p1v[:],
                                             func=AF.Relu,
                                             bias=bs[:, bs1:bs1 + 1],
                                             scale=1.0)
                        # ---- L2: K=128 f32r ----
                        ps2 = ps.tile([128, 4, NTILE], F32, tag="ps")
                        for q in range(4):
                            nc.tensor.matmul(
                                ps2[:, q, :],
                                wts[:, w2_c:w2_c + 128],
                                h1[:, q, :],
                                start=True, stop=True,
                            )
                        h2 = hbuf.tile([128, 4, NTILE], BF16, tag="h2")
                        p2v = ps2.rearrange("p a b -> p (a b)")
                        h2v = h2.rearrange("p a b -> p (a b)")
                        if it % 3 == 0:
                            nc.scalar.activation(out=h2v[:], in_=p2v[:],
                                                 func=AF.Relu,
                                                 bias=bs[:, bs2:bs2 + 1],
                                                 scale=1.0)
                        else:
                            nc.vector.tensor_scalar(
                                out=h2v[:], in0=p2v[:],
                                scalar1=bs[:, bs2:bs2 + 1], scalar2=0.0,
                                op0=ALU.add, op1=ALU.max)
                        # ---- L3: bf16, pairs (q0,q1)->J0, (q2,q3)->J1 ----
                        nb = NTILE // nseg  # quads per tile: 4 track, 8 robot
                        ps3 = ps.tile([128, 2, nb, nseg], F32, tag="ps")
                        p3v = ps3.rearrange("p a b c -> p (a b c)")
                        for q in range(4):
                            J, blk = q // 2, q % 2
                            nc.tensor.matmul(
                                p3v[64 * blk:64 * blk + 64,
                                    J * NTILE:(J + 1) * NTILE],
                                wbf[:, w3_c:w3_c + 64],
                                h2[:, q, :],
                                start=True, stop=True,
                                tile_position=(0, 64 * blk),
                            )
                        # ---- pooling reduces over nseg ----
                        p3r = ps3.rearrange("p a b c -> p (a b) c")
                        base = (32 * ch + 4 * t) if nseg == NT \
                            else (64 * ch + 8 * t)
                        sview = esum.rearrange("p (J r) -> p J r", J=2)[
                            :, :, base:base + nb]
                        mview = emax.rearrange("p (J r) -> p J r", J=2)[
                            :, :, base:base + nb]
                        nc.vector.reduce_sum(out=sview, in_=p3r[:], axis=AX.X)
                        nc.vector.reduce_max(out=mview, in_=p3r[:], axis=AX.X)

            branch(xt_d, QT, 2 * DT, W1T_C, W2T_C, 0, BS_TB1, BS_TB2, NT,
                   esum_t, emax_t)
            branch(xr_d, QR, 2 * DR, W1R_C, W2R_C, 64, BS_RB1, BS_RB2, NR,
                   esum_r, emax_r)

            # ---- combine: emb = esum/(2*nseg) + 0.5*emax  (bias folded
            #      into head mb1 on host) ----
            emb_t = acc.tile([128, 512], F32R, tag="emb_t")
            emb_r = acc.tile([128, 512], F32R, tag="emb_r")
            for esum, emax, emb, nseg in (
                    (esum_t, emax_t, emb_t, NT), (esum_r, emax_r, emb_r, NR)):
                tmp = hbuf.tile([128, 512], F32, tag="tmp")
                nc.vector.tensor_scalar(out=tmp[:], in0=esum[:],
                                        scalar1=1.0 / (2.0 * nseg),
                                        scalar2=None, op0=ALU.mult)
                nc.vector.scalar_tensor_tensor(
                    out=emb[:], in0=emax[:], scalar=0.5, in1=tmp[:],
                    op0=ALU.mult, op1=ALU.add)

            # ---- scatter emb -> cT rows: [44:76] robot, [76:108] track ----
            # block (J, blk, h): emb[64*blk+32*h : +32, 256J : 256J+256]
            #   -> cT[row0:row0+32, 1024h + 512J + 256blk : +256]
            for emb, row0 in ((emb_r, 44), (emb_t, 76)):
                for J in range(2):
                    for blk in range(2):
                        for h in range(2):
                            nc.sync.dma_start(
                                out=cT[row0:row0 + 32,
                                       1024 * h + 512 * J + 256 * blk:
                                       1024 * h + 512 * J + 256 * blk + 256],
                                in_=emb[64 * blk + 32 * h:
                                        64 * blk + 32 * h + 32,
                                        256 * J:256 * J + 256])

            # ---- head MLP 108 -> 128 -> 128 -> 64 -> 1 ----
            y_sb = acc.tile([1, B_C], F32, tag="y")
            for t in range(B_C // NTILE):
                cs = slice(t * NTILE, (t + 1) * NTILE)
                psA = ps.tile([128, NTILE], F32, tag="ps")
                nc.tensor.matmul(psA[:], wts[0:108, MW1_C:MW1_C + 128],
                                 cT[:, cs], start=True, stop=True)
                hh1 = head.tile([128, NTILE], F32R, tag="hh1")
                nc.scalar.activation(out=hh1[:], in_=psA[:], func=AF.Relu,
                                     bias=bs[:, BS_MB1:BS_MB1 + 1], scale=1.0)
                psB = ps.tile([128, NTILE], F32, tag="ps")
                nc.tensor.matmul(psB[:], wts[:, MW2_C:MW2_C + 128],
                                 hh1[:], start=True, stop=True)
                hh2 = head.tile([128, NTILE], F32R, tag="hh2")
                nc.scalar.activation(out=hh2[:], in_=psB[:], func=AF.Relu,
                                     bias=bs[:, BS_MB2:BS_MB2 + 1], scale=1.0)
                psC = ps.tile([64, NTILE], F32, tag="ps")
                nc.tensor.matmul(psC[:], wts[:, MW3_C:MW3_C + 64],
                                 hh2[:], start=True, stop=True)
                hh3 = head.tile([64, NTILE], F32R, tag="hh3")
                nc.scalar.activation(out=hh3[:], in_=psC[:], func=AF.Relu,
                                     bias=bs[0:64, BS_MB3:BS_MB3 + 1],
                                     scale=1.0)
                psD = ps.tile([1, NTILE], F32, tag="ps")
                nc.tensor.matmul(psD[:], wts[0:64, MW4_C:MW4_C + 1],
                                 hh3[:], start=True, stop=True)
                nc.scalar.activation(out=y_sb[:, cs], in_=psD[:],
                                     func=AF.Identity,
                                     bias=bs[0:1, BS_MB4:BS_MB4 + 1],
                                     scale=1.0)
            nc.sync.dma_start(out=y_d[:], in_=y_sb[:])

    nc.compile()
    return nc


def _pack_x(x, d, qcols):
    """x [rows, d] (rows = B_c*nseg, b-major) -> [128, qcols] with 4
    row-groups at partition offsets {0,32,64,96}; 2-row packing pairs
    row r with row r + rows/2."""
    rows = x.shape[0]
    half = rows // 2
    packed = np.concatenate([x[:half].T, x[half:].T], axis=0)  # [2d, half]
    out = np.zeros((128, qcols), dtype=np.float32)
    for q in range(4):
        out[32 * q:32 * q + 2 * d] = packed[:, q * qcols:(q + 1) * qcols]
    return np.ascontiguousarray(out)


def _blockdiag2(w):
    """w [d, m] -> [2d, 2m] block-diagonal."""
    d, m = w.shape
    out = np.zeros((2 * d, 2 * m), dtype=np.float32)
    out[:d, :m] = w
    out[d:, m:] = w
    return out


def _build_consts(i):
    np32 = lambda a: np.asarray(a, dtype=np.float32)
    wts = np.zeros((128, WTS_W), dtype=np.float32)
    # L1 lhsT blocks replicated at the 4 row-group offsets
    bd1t = _blockdiag2(np32(i["tw1"]))   # [14, 128]
    bd1r = _blockdiag2(np32(i["rw1"]))   # [12, 128]
    for q in range(4):
        wts[32 * q:32 * q + 14, W1T_C:W1T_C + 128] = bd1t
        wts[32 * q:32 * q + 12, W1R_C:W1R_C + 128] = bd1r
    wts[:, W2T_C:W2T_C + 128] = _blockdiag2(np32(i["tw2"]))
    wts[:, W2R_C:W2R_C + 128] = _blockdiag2(np32(i["rw2"]))
    wts[0:108, MW1_C:MW1_C + 128] = np32(i["mw1"])
    wts[:, MW2_C:MW2_C + 128] = np32(i["mw2"])
    wts[:, MW3_C:MW3_C + 64] = np32(i["mw3"])
    wts[0:64, MW4_C:MW4_C + 1] = np32(i["mw4"])

    wbf = np.zeros((128, 128), dtype=np.float32)
    wbf[:, 0:64] = _blockdiag2(np32(i["tw3"]))
    wbf[:, 64:128] = _blockdiag2(np32(i["rw3"]))
    wbf = wbf.astype(ml_dtypes.bfloat16)

    bs = np.zeros((128, 8), dtype=np.float32)
    bs[:, BS_TB1] = np.concatenate([np32(i["tb1"]), np32(i["tb1"])])
    bs[:, BS_RB1] = np.concatenate([np32(i["rb1"]), np32(i["rb1"])])
    bs[:, BS_TB2] = np.concatenate([np32(i["tb2"]), np32(i["tb2"])])
    bs[:, BS_RB2] = np.concatenate([np32(i["rb2"]), np32(i["rb2"])])
    # fold pooled e-bias into head L1 bias: c@mw1 picks up b3@mw1 rows
    mb1p = (np32(i["mb1"])
            + np32(i["rb3"]) @ np32(i["mw1"])[44:76]
            + np32(i["tb3"]) @ np32(i["mw1"])[76:108])
    bs[:, BS_MB1] = mb1p
    bs[:, BS_MB2] = np32(i["mb2"])
    bs[0:64, BS_MB3] = np32(i["mb3"])
    bs[0:1, BS_MB4] = np32(i["mb4"])
    return wts, wbf, bs


def kernel(**inputs) -> np.ndarray:
    if "nc" not in _CACHE:
        _CACHE["nc"] = _build_bass()
    nc = _CACHE["nc"]

    wts, wbf, bs = _build_consts(inputs)
    t0 = np.asarray(inputs["tier0_features"], dtype=np.float32)
    rb = np.asarray(inputs["robot_features"], dtype=np.float32)
    tk = np.asarray(inputs["track_features"], dtype=np.float32)

    in_maps = []
    for c in range(N_CORES):
        s = slice(c * B_C, (c + 1) * B_C)
        in_maps.append({
            "xt": _pack_x(tk[s].reshape(B_C * NT, DT), DT, QT),
            "xr": _pack_x(rb[s].reshape(B_C * NR, DR), DR, QR),
            "t0": np.ascontiguousarray(t0[s].T),
            "wts": wts, "wbf": wbf, "bs": bs,
        })

    res = run_bass_kernel_spmd(nc, in_maps, core_ids=list(range(N_CORES)))
    out = np.concatenate([r["y"][0] for r in res.results])
    return out.astype(np.float32)


if __name__ == "__main__":
    rng = np.random.default_rng(0)
    fake = {
        "tier0_features": rng.standard_normal((B, 44), dtype=np.float32),
        "robot_features": rng.standard_normal((B, NR, DR), dtype=np.float32),
        "track_features": rng.standard_normal((B, NT, DT), dtype=np.float32),
    }
    for n, sh in (("rw1", (6, 64)), ("rw2", (64, 64)), ("rw3", (64, 32)),
                  ("tw1", (7, 64)), ("tw2", (64, 64)), ("tw3", (64, 32)),
                  ("mw1", (108, 128)), ("mw2", (128, 128)),
                  ("mw3", (128, 64)), ("mw4", (64, 1))):
        fake[n] = rng.standard_normal(sh, dtype=np.float32) * 0.2
    for n, sh in (("rb1", 64), ("rb2", 64), ("rb3", 32),
                  ("tb1", 64), ("tb2", 64), ("tb3", 32),
                  ("mb1", 128), ("mb2", 128), ("mb3", 64), ("mb4", 1)):
        fake[n] = rng.standard_normal((sh,), dtype=np.float32) * 0.1
    y = kernel(**fake)
    print("kernel out:", y.shape, y[:4])


# revision 10
# speedup vs baseline: 8.0394x; 8.0394x over previous
"""Trainium2 Bass kernel for nn_CentralizedCritic (pooling critic net).

Data-parallel over 8 NeuronCores: each core handles B_c=2048 batch rows.

Per-core math (matches the jax reference):
  robot_emb = setenc(robot[b], rw*)  -> [B,32]   (mean+max pool over 64)
  track_emb = setenc(track[b], tw*)  -> [B,32]   (mean+max pool over 128)
  c = [tier0, robot_emb, track_emb]  -> [B,108]
  y = mlp(c)                         -> [B]

On-chip mapping:
  - Activations kept transposed [feat, rows]; 2 batch-halves packed on the
    partition dim via block-diag weights (K=2*d_in, M=2*d_hidden=128).
  - x^T pre-packed on host into 4 row-groups at partition offsets {0,32,64,96}
    so L1 matmuls row-tile the PE array.
  - L1/L2/head matmuls in float32r (full-rate fp32), L3 in bf16 so two
    column-tiles can pair into one [128,*] PSUM tile (halves pooling cost).
  - relu+bias fused into the PSUM->SBUF evacuation (ACT or DVE).
  - mean-pool: DVE reduce_sum from PSUM; max-pool: DVE reduce_max from PSUM.
  - e-bias (rb3/tb3) folded into the head-L1 bias on host.
"""

import sys

sys.path.insert(0, "/opt/trn_rl_repo")

import numpy as np
import ml_dtypes

import concourse.bass as bass  # noqa: F401  (bass must import before tile)
import concourse.mybir as mybir
import concourse.tile as tile
from concourse import bacc
from concourse.bass_utils import run_bass_kernel_spmd

F32R = mybir.dt.float32r
F32 = mybir.dt.float32
BF16 = mybir.dt.bfloat16
AF = mybir.ActivationFunctionType
ALU = mybir.AluOpType
AX = mybir.AxisListType

N_CORES = 8
B = 16384
B_C = B // N_CORES          # 2048 batch rows per core
HALF = B_C // 2             # 1024 (2-row packing pairs b and b+HALF)
NR, DR = 64, 6              # robot set size / feature dim
NT, DT = 128, 7             # track set size / feature dim
CT = B_C * NT // 2          # 131072 packed track cols per core
CR = B_C * NR // 2          # 65536 packed robot cols per core
QT = CT // 4                # 32768 cols per track row-group
QR = CR // 4                # 16384 cols per robot row-group
CHUNK = 4096                # dma chunk cols
NTILE = 512                 # matmul free dim

# const-block column layout in "wts" [128, 840] (f32r)
W1T_C, W1R_C, W2T_C, W2R_C = 0, 128, 256, 384
MW1_C, MW2_C, MW3_C, MW4_C = 512, 640, 768, 832
WTS_W = 840
# "bs" [128, 8] (f32) bias columns
BS_TB1, BS_RB1, BS_TB2, BS_RB2, BS_MB1, BS_MB2, BS_MB3, BS_MB4 = range(8)

_CACHE = {}

import os
PROBE_NO_REDUCE = os.environ.get("PROBE_NO_REDUCE") == "1"
PROBE_NO_EVAC = os.environ.get("PROBE_NO_EVAC") == "1"
PROBE_NO_MM3 = os.environ.get("PROBE_NO_MM3") == "1"
HBUF_BUFS = int(os.environ.get("HBUF_BUFS", "3"))
PS_BUFS = int(os.environ.get("PS_BUFS", "4"))
DVE_EVAC_MOD = int(os.environ.get("DVE_EVAC_MOD", "3"))
MMDT_NAME = os.environ.get("MMDT", "f32r")
MMDT = mybir.dt.bfloat16 if MMDT_NAME == "bf16" else mybir.dt.float32r
MMDT_NP = ml_dtypes.bfloat16 if MMDT_NAME == "bf16" else np.float32


def _build_bass():
    nc = bacc.Bacc("TRN2", target_bir_lowering=False, debug=False,
                   num_devices=N_CORES)
    xt_d = nc.dram_tensor("xt", [128, QT], MMDT, kind="ExternalInput")
    xr_d = nc.dram_tensor("xr", [128, QR], MMDT, kind="ExternalInput")
    t0_d = nc.dram_tensor("t0", [44, B_C], MMDT, kind="ExternalInput")
    wts_d = nc.dram_tensor("wts", [128, WTS_W], MMDT, kind="ExternalInput")
    wbf_d = nc.dram_tensor("wbf", [128, 128], BF16, kind="ExternalInput")
    bs_d = nc.dram_tensor("bs", [128, 8], F32, kind="ExternalInput")
    y_d = nc.dram_tensor("y", [1, B_C], F32, kind="ExternalOutput")

    with tile.TileContext(nc) as tc:
        with (
            tc.tile_pool(name="consts", bufs=1) as consts,
            tc.tile_pool(name="xchunks", bufs=3) as xchunks,
            tc.tile_pool(name="hbuf", bufs=HBUF_BUFS) as hbuf,
            tc.tile_pool(name="acc", bufs=1) as acc,
            tc.tile_pool(name="head", bufs=2) as head,
            tc.tile_pool(name="ps", bufs=PS_BUFS, space="PSUM") as ps,
        ):
            wts = consts.tile([128, WTS_W], MMDT)
            wbf = consts.tile([128, 128], BF16)
            bs = consts.tile([128, 8], F32)
            nc.sync.dma_start(out=wts[:], in_=wts_d[:])
            nc.sync.dma_start(out=wbf[:], in_=wbf_d[:])
            nc.sync.dma_start(out=bs[:], in_=bs_d[:])

            cT = acc.tile([108, B_C], MMDT)
            nc.sync.dma_start(out=cT[0:44, :], in_=t0_d[:])

            # pooled accumulators: col = 256*J + 32*ch + 4*t + om (track)
            #                      col = 256*J + 64*ch + 8*t + om (robot)
            esum_t = acc.tile([128, 512], F32, tag="esum_t")
            emax_t = acc.tile([128, 512], F32, tag="emax_t")
            esum_r = acc.tile([128, 512], F32, tag="esum_r")
            emax_r = acc.tile([128, 512], F32, tag="emax_r")

            it_counter = [0]
            evac_counter = [0]

            def branch(x_d, qcols, k2, w1_c, w2_c, w3_c, bs1, bs2, nseg,
                       esum, emax):
                """One set-encoder branch. k2 = packed K (14 track / 12 robot).
                nseg = set size (128 track / 64 robot)."""
                nchunks = qcols // CHUNK
                tpc = CHUNK // NTILE  # tiles per chunk per group = 8
                for ch in range(nchunks):
                    xc = xchunks.tile([128, CHUNK], MMDT, tag="xc")
                    nc.sync.dma_start(
                        out=xc[:], in_=x_d[:, ch * CHUNK:(ch + 1) * CHUNK])
                    for t in range(tpc):
                        it = it_counter[0]
                        it_counter[0] += 1
                        cs = slice(t * NTILE, (t + 1) * NTILE)
                        h1 = hbuf.tile([128, 4, NTILE], MMDT, tag="h1")
                        h2 = hbuf.tile([128, 4, NTILE], BF16, tag="h2")

                        def evac(pshalf, htile, half, bias_col):
                            """relu+bias PSUM->SBUF for q-pair `half`;
                            engine round-robins, ~1/4 to DVE."""
                            pv = pshalf.rearrange("p a b -> p (a b)")
                            hv = htile[:, 2 * half:2 * half + 2, :].rearrange(
                                "p a b -> p (a b)")
                            k = evac_counter[0]
                            evac_counter[0] += 1
                            if k % DVE_EVAC_MOD == DVE_EVAC_MOD - 1:
                                nc.vector.tensor_scalar(
                                    out=hv[:], in0=pv[:],
                                    scalar1=bs[:, bias_col:bias_col + 1],
                                    scalar2=0.0, op0=ALU.add, op1=ALU.max)
                            else:
                                nc.scalar.activation(
                                    out=hv[:], in_=pv[:], func=AF.Relu,
                                    bias=bs[:, bias_col:bias_col + 1],
                                    scale=1.0)

                        for half in range(2):
                            # ---- L1: 2 row-tiled matmuls (K=k2 each) ----
                            ps1 = ps.tile([128, 2, NTILE], F32, tag="ps")
                            for j in range(2):
                                q = 2 * half + j
                                nc.tensor.matmul(
                                    ps1[:, j, :],
                                    wts[32 * q:32 * q + k2, w1_c:w1_c + 128],
                                    xc[32 * q:32 * q + k2, cs],
                                    start=True, stop=True,
                                    tile_position=(32 * q, 0),
                                )
                            evac(ps1, h1, half, bs1)
                            # ---- L2: K=128 f32r ----
                            ps2 = ps.tile([128, 2, NTILE], F32, tag="ps")
                            for j in range(2):
                                q = 2 * half + j
                                nc.tensor.matmul(
                                    ps2[:, j, :],
                                    wts[:, w2_c:w2_c + 128],
                                    h1[:, q, :],
                                    start=True, stop=True,
                                )
                            evac(ps2, h2, half, bs2)
                        # ---- L3: bf16, pairs (q0,q1)->J0, (q2,q3)->J1 ----
                        nb = NTILE // nseg  # quads per tile: 4 track, 8 robot
                        ps3 = ps.tile([128, 2 * nb * nseg], F32, tag="ps")
                        ps3 = ps3.rearrange("p (a b c) -> p a b c", a=2, b=nb)
                        p3v = ps3.rearrange("p a b c -> p (a b c)")
                        for q in range(4):
                            J, blk = q // 2, q % 2
                            nc.tensor.matmul(
                                p3v[64 * blk:64 * blk + 64,
                                    J * NTILE:(J + 1) * NTILE],
                                wbf[:, w3_c:w3_c + 64],
                                h2[:, q, :],
                                start=True, stop=True,
                                tile_position=(0, 64 * blk),
                            )
                        # ---- pooling reduces over nseg ----
                        p3r = ps3.rearrange("p a b c -> p (a b) c")
                        base = (32 * ch + 4 * t) if nseg == NT \
                            else (64 * ch + 8 * t)
                        sview = esum.rearrange("p (J r) -> p J r", J=2)[
                            :, :, base:base + nb]
                        mview = emax.rearrange("p (J r) -> p J r", J=2)[
                            :, :, base:base + nb]
                        if not PROBE_NO_REDUCE:
                            nc.vector.reduce_sum(out=sview, in_=p3r[:],
                                                 axis=AX.X)
                            nc.vector.reduce_max(out=mview, in_=p3r[:],
                                                 axis=AX.X)

            branch(xt_d, QT, 2 * DT, W1T_C, W2T_C, 0, BS_TB1, BS_TB2, NT,
                   esum_t, emax_t)
            branch(xr_d, QR, 2 * DR, W1R_C, W2R_C, 64, BS_RB1, BS_RB2, NR,
                   esum_r, emax_r)

            # ---- combine: emb = esum/(2*nseg) + 0.5*emax  (bias folded
            #      into head mb1 on host) ----
            emb_t = acc.tile([128, 512], MMDT, tag="emb_t")
            emb_r = acc.tile([128, 512], MMDT, tag="emb_r")
            for esum, emax, emb, nseg in (
                    (esum_t, emax_t, emb_t, NT), (esum_r, emax_r, emb_r, NR)):
                tmp = hbuf.tile([128, 512], F32, tag="tmp")
                nc.vector.tensor_scalar(out=tmp[:], in0=esum[:],
                                        scalar1=1.0 / (2.0 * nseg),
                                        scalar2=None, op0=ALU.mult)
                nc.vector.scalar_tensor_tensor(
                    out=emb[:], in0=emax[:], scalar=0.5, in1=tmp[:],
                    op0=ALU.mult, op1=ALU.add)

            # ---- scatter emb -> cT rows: [44:76] robot, [76:108] track ----
            # block (J, blk, h): emb[64*blk+32*h : +32, 256J : 256J+256]
            #   -> cT[row0:row0+32, 1024h + 512J + 256blk : +256]
            for emb, row0 in ((emb_r, 44), (emb_t, 76)):
                for J in range(2):
                    for blk in range(2):
                        for h in range(2):
                            nc.sync.dma_start(
                                out=cT[row0:row0 + 32,
                                       1024 * h + 512 * J + 256 * blk:
                                       1024 * h + 512 * J + 256 * blk + 256],
                                in_=emb[64 * blk + 32 * h:
                                        64 * blk + 32 * h + 32,
                                        256 * J:256 * J + 256])

            # ---- head MLP 108 -> 128 -> 128 -> 64 -> 1 ----
            y_sb = acc.tile([1, B_C], F32, tag="y")
            for t in range(B_C // NTILE):
                cs = slice(t * NTILE, (t + 1) * NTILE)
                psA = ps.tile([128, NTILE], F32, tag="ps")
                nc.tensor.matmul(psA[:], wts[0:108, MW1_C:MW1_C + 128],
                                 cT[:, cs], start=True, stop=True)
                hh1 = head.tile([128, NTILE], MMDT, tag="hh1")
                nc.scalar.activation(out=hh1[:], in_=psA[:], func=AF.Relu,
                                     bias=bs[:, BS_MB1:BS_MB1 + 1], scale=1.0)
                psB = ps.tile([128, NTILE], F32, tag="ps")
                nc.tensor.matmul(psB[:], wts[:, MW2_C:MW2_C + 128],
                                 hh1[:], start=True, stop=True)
                hh2 = head.tile([128, NTILE], MMDT, tag="hh2")
                nc.scalar.activation(out=hh2[:], in_=psB[:], func=AF.Relu,
                                     bias=bs[:, BS_MB2:BS_MB2 + 1], scale=1.0)
                psC = ps.tile([64, NTILE], F32, tag="ps")
                nc.tensor.matmul(psC[:], wts[:, MW3_C:MW3_C + 64],
                                 hh2[:], start=True, stop=True)
                hh3 = head.tile([64, NTILE], MMDT, tag="hh3")
                nc.scalar.activation(out=hh3[:], in_=psC[:], func=AF.Relu,
                                     bias=bs[0:64, BS_MB3:BS_MB3 + 1],
                                     scale=1.0)
                psD = ps.tile([1, NTILE], F32, tag="ps")
                nc.tensor.matmul(psD[:], wts[0:64, MW4_C:MW4_C + 1],
                                 hh3[:], start=True, stop=True)
                nc.scalar.activation(out=y_sb[:, cs], in_=psD[:],
                                     func=AF.Identity,
                                     bias=bs[0:1, BS_MB4:BS_MB4 + 1],
                                     scale=1.0)
            nc.sync.dma_start(out=y_d[:], in_=y_sb[:])

    nc.compile()
    return nc


def _pack_x(x, d, qcols):
    """x [rows, d] (rows = B_c*nseg, b-major) -> [128, qcols] with 4
    row-groups at partition offsets {0,32,64,96}; 2-row packing pairs
    row r with row r + rows/2."""
    rows = x.shape[0]
    half = rows // 2
    packed = np.concatenate([x[:half].T, x[half:].T], axis=0)  # [2d, half]
    out = np.zeros((128, qcols), dtype=MMDT_NP)
    for q in range(4):
        out[32 * q:32 * q + 2 * d] = packed[:, q * qcols:(q + 1) * qcols]
    return np.ascontiguousarray(out)


def _blockdiag2(w):
    """w [d, m] -> [2d, 2m] block-diagonal."""
    d, m = w.shape
    out = np.zeros((2 * d, 2 * m), dtype=np.float32)
    out[:d, :m] = w
    out[d:, m:] = w
    return out


def _build_consts(i):
    np32 = lambda a: np.asarray(a, dtype=np.float32)
    wts = np.zeros((128, WTS_W), dtype=np.float32)
    # L1 lhsT blocks replicated at the 4 row-group offsets
    bd1t = _blockdiag2(np32(i["tw1"]))   # [14, 128]
    bd1r = _blockdiag2(np32(i["rw1"]))   # [12, 128]
    for q in range(4):
        wts[32 * q:32 * q + 14, W1T_C:W1T_C + 128] = bd1t
        wts[32 * q:32 * q + 12, W1R_C:W1R_C + 128] = bd1r
    wts[:, W2T_C:W2T_C + 128] = _blockdiag2(np32(i["tw2"]))
    wts[:, W2R_C:W2R_C + 128] = _blockdiag2(np32(i["rw2"]))
    wts[0:108, MW1_C:MW1_C + 128] = np32(i["mw1"])
    wts[:, MW2_C:MW2_C + 128] = np32(i["mw2"])
    wts[:, MW3_C:MW3_C + 64] = np32(i["mw3"])
    wts[0:64, MW4_C:MW4_C + 1] = np32(i["mw4"])
    wts = wts.astype(MMDT_NP)

    wbf = np.zeros((128, 128), dtype=np.float32)
    wbf[:, 0:64] = _blockdiag2(np32(i["tw3"]))
    wbf[:, 64:128] = _blockdiag2(np32(i["rw3"]))
    wbf = wbf.astype(ml_dtypes.bfloat16)

    bs = np.zeros((128, 8), dtype=np.float32)
    bs[:, BS_TB1] = np.concatenate([np32(i["tb1"]), np32(i["tb1"])])
    bs[:, BS_RB1] = np.concatenate([np32(i["rb1"]), np32(i["rb1"])])
    bs[:, BS_TB2] = np.concatenate([np32(i["tb2"]), np32(i["tb2"])])
    bs[:, BS_RB2] = np.concatenate([np32(i["rb2"]), np32(i["rb2"])])
    # fold pooled e-bias into head L1 bias: c@mw1 picks up b3@mw1 rows
    mb1p = (np32(i["mb1"])
            + np32(i["rb3"]) @ np32(i["mw1"])[44:76]
            + np32(i["tb3"]) @ np32(i["mw1"])[76:108])
    bs[:, BS_MB1] = mb1p
    bs[:, BS_MB2] = np32(i["mb2"])
    bs[0:64, BS_MB3] = np32(i["mb3"])
    bs[0:1, BS_MB4] = np32(i["mb4"])
    return wts, wbf, bs


def kernel(**inputs) -> np.ndarray:
    if "nc" not in _CACHE:
        _CACHE["nc"] = _build_bass()
    nc = _CACHE["nc"]

    wts, wbf, bs = _build_consts(inputs)
    t0 = np.asarray(inputs["tier0_features"], dtype=np.float32)
    rb = np.asarray(inputs["robot_features"], dtype=np.float32)
    tk = np.asarray(inputs["track_features"], dtype=np.float32)

    in_maps = []
    for c in range(N_CORES):
        s = slice(c * B_C, (c + 1) * B_C)
        in_maps.append({
            "xt": _pack_x(tk[s].reshape(B_C * NT, DT), DT, QT),
            "xr": _pack_x(rb[s].reshape(B_C * NR, DR), DR, QR),
            "t0": np.ascontiguousarray(t0[s].T).astype(MMDT_NP),
            "wts": wts, "wbf": wbf, "bs": bs,
        })

    res = run_bass_kernel_spmd(nc, in_maps, core_ids=list(range(N_CORES)))
    out = np.concatenate([r["y"][0] for r in res.results])
    return out.astype(np.float32)


if __name__ == "__main__":
    rng = np.random.default_rng(0)
    fake = {
        "tier0_features": rng.standard_normal((B, 44), dtype=np.float32),
        "robot_features": rng.standard_normal((B, NR, DR), dtype=np.float32),
        "track_features": rng.standard_normal((B, NT, DT), dtype=np.float32),
    }
    for n, sh in (("rw1", (6, 64)), ("rw2", (64, 64)), ("rw3", (64, 32)),
                  ("tw1", (7, 64)), ("tw2", (64, 64)), ("tw3", (64, 32)),
                  ("mw1", (108, 128)), ("mw2", (128, 128)),
                  ("mw3", (128, 64)), ("mw4", (64, 1))):
        fake[n] = rng.standard_normal(sh, dtype=np.float32) * 0.2
    for n, sh in (("rb1", 64), ("rb2", 64), ("rb3", 32),
                  ("tb1", 64), ("tb2", 64), ("tb3", 32),
                  ("mb1", 128), ("mb2", 128), ("mb3", 64), ("mb4", 1)):
        fake[n] = rng.standard_normal((sh,), dtype=np.float32) * 0.1
    y = kernel(**fake)
    print("kernel out:", y.shape, y[:4])
